# revision 10
# baseline (speedup 1.0000x reference)
"""MiniMaxText01 linear attention layer on 8 Trainium2 NeuronCores.

Tensor-parallel over heads (4 heads per core). Per core:
  - fused QKV+gate projection in transposed layout (features on
    partitions, sequence on free dim), bf16 matmuls into 2-bank PSUM
    tiles (two m-tiles per tile) so the SiLU/Tanh drains run as one
    [128,1024] ACT op each; gate sigmoid computed as 0.5*(1+tanh(x/2))
    so every ACT function (Silu/Tanh/Square/Copy) shares one LUT set
  - lightning (chunked linear) attention with per-head decay, two heads
    packed per 128-partition group; per block the k'/v PE-transposes
    land in ONE [128,512] PSUM bank drained by a single DVE copy, and
    both head-pairs' attention outputs land in ONE [128,512] bank so
    the square/gate-mul drains are single 512-col ops
  - RMSNorm variance: ones-matmul partition reduction, per-chunk 2KB
    AllReduce across the 8 cores, DVE-only Newton rsqrt; out-projection
    for chunk c runs LAG chunks behind the attention pipeline so the
    collective latency is hidden
  - out-proj row-parallel: each core emits a full-width partial output
    (transposed, bf16); host sums the 8 partials and transposes back.

Everything is hardcoded for the fixed problem shapes below.
"""

import math
import warnings

warnings.filterwarnings("ignore")

import numpy as np
import ml_dtypes

import concourse.bacc as bacc
import concourse.mybir as mybir
import concourse.tile as tile
from concourse.bass_utils import run_bass_kernel_spmd
from concourse.masks import make_identity

F32 = mybir.dt.float32
I32 = mybir.dt.int32
BF16 = mybir.dt.bfloat16
BF = ml_dtypes.bfloat16
AF = mybir.ActivationFunctionType
ALU = mybir.AluOpType

N = 8192          # sequence length
HID = 2048        # hidden size
H = 32            # total heads
D = 64            # head dim
BLOCK = 256       # attention chunk size
NCORES = 8
HL = H // NCORES  # 4 local heads per core
CHUNK = 512       # seq columns processed per projection chunk
NCHUNK = N // CHUNK
BPC = CHUNK // BLOCK  # blocks per chunk
EPS = 1e-5
NUM_LAYERS, LAYER_IDX = 80, 0
MAGIC = 0x5F3759DF

LAST_EXEC_NS = None
LAST_RESULTS = None


def _build_slopes(n):
    def p2(m):
        start = 2 ** (-(2 ** (-(math.log2(m) - 3))))
        return [start * start**i for i in range(m)]

    if math.log2(n).is_integer():
        s = p2(n)
    else:
        cp = 2 ** math.floor(math.log2(n))
        s = p2(cp) + _build_slopes(2 * cp).tolist()[0::2][: n - cp]
    return np.array(s, dtype=np.float32)


SLOPE = _build_slopes(H) * (1.0 - LAYER_IDX / (NUM_LAYERS - 1) + 1e-5)  # [H]

_NC_CACHE = None


DEFAULT_OPTS = dict(
    psA=3, psB=2, psC=1, psV=2,
    hstb=4, combb=3, gateb=3, ghb=5, kvb=4, kptb=3, qpb=3, qkpb=3, hsqb=3, osbb=6,
    lag=2, cc_stride=1, rbcb=2, rcpb=2, warmup=48,
)


def _build_module(**opts):
    o = dict(DEFAULT_OPTS)
    o.update(opts)
    nc = bacc.Bacc("TRN2", target_bir_lowering=False, num_devices=NCORES)

    hsT_d = nc.dram_tensor("hsT", [HID, N], BF16, kind="ExternalInput")
    wc_d = nc.dram_tensor("wcomb", [8, 128, HID], BF16, kind="ExternalInput")
    wo_d = nc.dram_tensor("wout", [2 * 128, HID], BF16, kind="ExternalInput")
    dd_d = nc.dram_tensor("dd", [128, 2 * HL, BLOCK], BF16, kind="ExternalInput")
    qd_d = nc.dram_tensor("qd", [128, HL // 2, BLOCK], BF16, kind="ExternalInput")
    kdb_d = nc.dram_tensor("kdb", [128, HL // 2, BLOCK], BF16, kind="ExternalInput")
    bd_d = nc.dram_tensor("bd", [128, HL // 2], F32, kind="ExternalInput")
    kv0_d = nc.dram_tensor("kv0", [128, HL // 2, D], F32, kind="ExternalInput")
    outp_d = nc.dram_tensor("outp", [HID, N], BF16, kind="ExternalOutput")

    with tile.TileContext(nc) as tc:
        with (
            tc.tile_pool(name="singles", bufs=1) as sg,
            tc.tile_pool(name="hstp", bufs=o["hstb"]) as hstp,
            tc.tile_pool(name="combp", bufs=o["combb"]) as combp,
            tc.tile_pool(name="gatep", bufs=o["gateb"]) as gatep,
            tc.tile_pool(name="ghp", bufs=o["ghb"]) as ghp,
            tc.tile_pool(name="kvp", bufs=o["kvb"]) as kvp,
            tc.tile_pool(name="kptp", bufs=o["kptb"]) as kptp,
            tc.tile_pool(name="qpp", bufs=o["qpb"]) as qpp,
            tc.tile_pool(name="qkpp", bufs=o["qkpb"]) as qkpp,
            tc.tile_pool(name="hsqp", bufs=o["hsqb"]) as hsqp,
            tc.tile_pool(name="osbp", bufs=o["osbb"]) as osbp,
            tc.tile_pool(name="rcp", bufs=o["rcpb"]) as rcp,
            tc.tile_pool(name="psA", bufs=o["psA"], space="PSUM") as psA,
            tc.tile_pool(name="psB", bufs=o["psB"], space="PSUM") as psB,
            tc.tile_pool(name="psC", bufs=o["psC"], space="PSUM") as psC,
            tc.tile_pool(name="psV", bufs=o["psV"], space="PSUM") as psV,
            tc.tile_pool(name="dram", bufs=1, space="DRAM") as dram,
        ):
            # ---- resident tensors -------------------------------------
            wcm = []
            for mt in range(8):
                wct = sg.tile([128, HID // 128, 128], BF16, name=f"wcm{mt}")
                wcm.append(wct)
            wo_sb = sg.tile([128, 2, HID], BF16)
            dd_sb = sg.tile([128, 2 * HL, BLOCK], BF16)
            qd_sb = sg.tile([128, HL // 2, BLOCK], BF16)
            kdb_sb = sg.tile([128, HL // 2, BLOCK], BF16)
            bd_sb = sg.tile([128, HL // 2], F32)
            S32 = sg.tile([128, HL // 2, D], F32)
            Sbf = sg.tile([128, HL // 2, D], BF16)
            ones_sb = sg.tile([128, 1], BF16)
            nc.vector.memset(ones_sb, 1.0)
            ident = sg.tile([128, 128], BF16)
            make_identity(nc, ident)

            cc_in = dram.tile([1, N], F32)
            cc_out = dram.tile([1, N], F32)
            r_dram = dram.tile([1, N], F32)

            hsT_r = hsT_d[:].rearrange("(kt p) s -> p kt s", p=128)

            ghts = {}

            def load_hst(c):
                C0 = c * CHUNK
                hst_lo = hstp.tile([128, HID // 256, CHUNK], BF16, name="hst_lo", tag="hst_lo")
                hst_hi = hstp.tile([128, HID // 256, CHUNK], BF16, name="hst_hi", tag="hst_hi")
                nc.sync.dma_start(out=hst_lo, in_=hsT_r[:, 0 : HID // 256, C0 : C0 + CHUNK])
                nc.sync.dma_start(out=hst_hi, in_=hsT_r[:, HID // 256 :, C0 : C0 + CHUNK])
                return hst_lo, hst_hi

            # chunk 0+1 activations and the first weight m-tiles go out
            # first (critical path of the first projection group); the rest
            # spreads over the scalar/gpsimd DMA queues.
            hst0 = load_hst(0)
            nc.sync.dma_start(
                out=wcm[0], in_=wc_d[0].rearrange("p (kt m) -> p kt m", m=128)
            )
            hst1 = load_hst(1)
            for mt in range(1, 8):
                eng = nc.scalar if mt < 5 else nc.gpsimd
                eng.dma_start(
                    out=wcm[mt], in_=wc_d[mt].rearrange("p (kt m) -> p kt m", m=128)
                )
            nc.gpsimd.dma_start(out=dd_sb, in_=dd_d[:])
            nc.gpsimd.dma_start(out=qd_sb, in_=qd_d[:])
            nc.gpsimd.dma_start(out=kdb_sb, in_=kdb_d[:])
            nc.gpsimd.dma_start(out=bd_sb, in_=bd_d[:])
            nc.gpsimd.dma_start(out=S32, in_=kv0_d[:])
            nc.vector.tensor_copy(Sbf[:], S32[:])

            # PE warm-up during the initial DMA: junk transposes so the HAM
            # clock-gate reaches 8/8 before the first real matmul.
            warm = psB.tile([128, 512], BF16, tag="qk", name="warm")
            for i in range(o["warmup"]):
                nc.tensor.transpose(
                    warm[:, (i % 4) * 128 : (i % 4 + 1) * 128], in_=ident, identity=ident
                )

            def emit_proj_sweep(chunks, hsts):
                # weight-reuse sweep: one LDWEIGHTS per (mt, kt) serves one
                # matmul per chunk (the stationary operand stays loaded)
                out = []
                for c in chunks:
                    comb = combp.tile([128, 6, CHUNK], BF16, name="comb")
                    gate = gatep.tile([128, 2, CHUNK], F32, name="gate")
                    out.append((comb, gate))
                for mt in range(8):
                    pjs = [psA.tile([128, CHUNK], F32, tag="pj", name="pj") for _ in chunks]
                    for kt in range(HID // 128):
                        for i in range(len(chunks)):
                            hst_lo, hst_hi = hsts[i]
                            hsth = hst_lo if kt < HID // 256 else hst_hi
                            nc.tensor.matmul(
                                pjs[i],
                                lhsT=wcm[mt][:, kt, :],
                                rhs=hsth[:, kt % (HID // 256), :],
                                start=(kt == 0),
                                stop=(kt == HID // 128 - 1),
                            )
                    for i in range(len(chunks)):
                        comb, gate = out[i]
                        if mt < 6:
                            nc.scalar.activation(comb[:, mt, :], pjs[i], AF.Silu)
                        else:
                            # sigmoid(x) = 0.5*(1 + tanh(x/2)); the 0.5 is
                            # folded into W_out on the host, +1 applied below.
                            nc.scalar.activation(gate[:, mt - 6, :], pjs[i], AF.Tanh, scale=0.5)
                for comb, gate in out:
                    nc.vector.tensor_scalar_add(gate[:], gate[:], 1.0)
                return out

            def emit_attention(c, comb, gate):
                C0 = c * CHUNK
                ght = ghp.tile([128, 2, CHUNK], BF16, name="ght", tag="ght")
                ghts[c] = ght

                ps_var = psV.tile([1, CHUNK], F32, tag="var", name="ps_var")

                for blk in range(BPC):
                    bc = blk * BLOCK
                    kvs = {}
                    # phase A: k-decay pre-scale + PE-transpose k', v into
                    # one [128,512] bf16 bank, single DVE drain
                    for p in range(2):
                        kpt = kptp.tile([128, BLOCK], BF16, tag="kpt", name="kpt")
                        nc.vector.tensor_mul(
                            kpt, comb[:, 2 + p, bc : bc + BLOCK], kdb_sb[:, p, :]
                        )
                        tp = psB.tile([128, 512], BF16, tag="qk", name="tp")
                        for half in range(2):
                            nc.tensor.transpose(
                                tp[:, half * 128 : (half + 1) * 128],
                                in_=kpt[:, half * 128 : (half + 1) * 128],
                                identity=ident,
                            )
                            nc.tensor.transpose(
                                tp[:, 256 + half * 128 : 256 + (half + 1) * 128],
                                in_=comb[:, 4 + p, bc + half * 128 : bc + (half + 1) * 128],
                                identity=ident,
                            )
                        kvt = kvp.tile([128, 4, 128], BF16, tag="kvt", name="kvt")
                        kvs[p] = kvt
                        nc.vector.tensor_copy(
                            kvt[:].rearrange("p a b -> p (a b)"), tp[:]
                        )
                    # phase B: decayed queries (both heads of a pair at once)
                    qps = {}
                    for p in range(2):
                        qp = qpp.tile([128, BLOCK], BF16, tag="qp", name="qp")
                        qps[p] = qp
                        nc.vector.tensor_mul(
                            qp, comb[:, p, bc : bc + BLOCK], qd_sb[:, p, :]
                        )
                    # phase C: scores (transposed) + decay mask
                    qkp = {}
                    for p in range(2):
                        for hi in range(2):
                            h = 2 * p + hi
                            b = hi * 64
                            qkph = qkpp.tile([128, 2, BLOCK], BF16, tag="qkp", name="qkph")
                            qkp[h] = qkph
                            qk_ps = psB.tile([128, 512], F32, tag="qk", name="qk_ps")
                            for half in range(2):
                                nc.tensor.matmul(
                                    qk_ps[:, half * 256 : (half + 1) * 256],
                                    lhsT=comb[b : b + 64, 2 + p, bc + half * 128 : bc + (half + 1) * 128],
                                    rhs=comb[b : b + 64, p, bc : bc + BLOCK],
                                    start=True,
                                    stop=True,
                                    tile_position=(b, 0),
                                )
                            nc.vector.tensor_mul(
                                qkph[:].rearrange("p a b -> p (a b)"),
                                qk_ps,
                                dd_sb[:, 2 * h : 2 * h + 2, :].rearrange("p a b -> p (a b)"),
                            )
                    # phase D: attention output (transposed): inter + intra,
                    # both head-pairs into one [128,512] bank
                    po = psC.tile([128, 2, BLOCK], F32, tag="po", name="po")
                    for p in range(2):
                        for hi in range(2):
                            b = hi * 64
                            nc.tensor.matmul(
                                po[b : b + 64, p, :],
                                lhsT=Sbf[b : b + 64, p, :],
                                rhs=qps[p][b : b + 64, :],
                                start=True,
                                stop=False,
                                tile_position=(b, b),
                            )
                        for hi in range(2):
                            b = hi * 64
                            for half in range(2):
                                nc.tensor.matmul(
                                    po[b : b + 64, p, :],
                                    lhsT=kvs[p][:, 2 + half, b : b + 64],
                                    rhs=qkp[2 * p + hi][:, half, :],
                                    start=False,
                                    stop=(half == 1),
                                    tile_position=(0, b),
                                )
                    # phase E: variance partials + gated hidden (single ops)
                    hsq = hsqp.tile([128, 2, BLOCK], BF16, tag="hsq", name="hsq")
                    nc.scalar.square(hsq[:].rearrange("p a b -> p (a b)"),
                                     po[:].rearrange("p a b -> p (a b)"))
                    for p in range(2):
                        nc.tensor.matmul(
                            ps_var[0:1, bc : bc + BLOCK],
                            lhsT=ones_sb,
                            rhs=hsq[:, p, :],
                            start=(p == 0),
                            stop=(p == 1),
                        )
                    nc.vector.tensor_mul(
                        ght[:, :, bc : bc + BLOCK],
                        po[:],
                        gate[:, :, bc : bc + BLOCK],
                    )
                    # phase F: state update S = bd*S + k'^T v
                    for p in range(2):
                        psS = psB.tile([128, D], F32, tag="qk", name="psS")
                        for hi in range(2):
                            b = hi * 64
                            for half in range(2):
                                nc.tensor.matmul(
                                    psS[b : b + 64, :],
                                    lhsT=kvs[p][:, half, b : b + 64],
                                    rhs=kvs[p][:, 2 + half, b : b + 64],
                                    start=(half == 0),
                                    stop=(half == 1),
                                    tile_position=(0, b),
                                )
                        nc.scalar.mul(S32[:, p, :], S32[:, p, :], bd_sb[:, p : p + 1])
                        nc.vector.tensor_add(S32[:, p, :], S32[:, p, :], psS)
                    nc.vector.tensor_copy(Sbf[:], S32[:])

                # variance all-reduce + Newton rsqrt (DVE only)
                ssqc = hsqp.tile([1, CHUNK], F32, tag="ssqc", name="ssqc")
                nc.scalar.copy(ssqc, ps_var)
                nc.scalar.dma_start(out=cc_in[0:1, C0 : C0 + CHUNK], in_=ssqc)
                stride = o["cc_stride"]
                if (c + 1) % stride == 0:
                    R0 = (c + 1 - stride) * CHUNK
                    RW = stride * CHUNK
                    nc.gpsimd.collective_compute(
                        "AllReduce",
                        mybir.AluOpType.add,
                        replica_groups=[list(range(NCORES))],
                        ins=[cc_in[0:1, R0 : R0 + RW].opt()],
                        outs=[cc_out[0:1, R0 : R0 + RW].opt()],
                    )
                    z = rcp.tile([128, RW // 128], F32, tag="z", name="z")
                    nc.gpsimd.dma_start(
                        out=z,
                        in_=cc_out[0:1, R0 : R0 + RW].rearrange("a (p j) -> (a p) j", p=128),
                    )
                    nc.vector.tensor_scalar(
                        out=z, in0=z, scalar1=1.0 / HID, scalar2=EPS, op0=ALU.mult, op1=ALU.add
                    )
                    yi = rcp.tile([128, RW // 128], I32, tag="yi", name="yi")
                    nc.vector.tensor_scalar(
                        out=yi, in0=z.bitcast(I32), scalar1=1, scalar2=None,
                        op0=ALU.logical_shift_right,
                    )
                    nc.vector.tensor_scalar(
                        out=yi, in0=yi, scalar1=-1, scalar2=MAGIC, op0=ALU.mult, op1=ALU.add
                    )
                    y = yi.bitcast(F32)
                    t = rcp.tile([128, RW // 128], F32, tag="t", name="t")
                    for _ in range(2):
                        nc.vector.tensor_mul(t, y, y)
                        nc.vector.tensor_mul(t, t, z)
                        nc.vector.tensor_scalar(
                            out=t, in0=t, scalar1=-0.5, scalar2=1.5, op0=ALU.mult, op1=ALU.add
                        )
                        nc.vector.tensor_mul(y, y, t)
                    nc.gpsimd.dma_start(
                        out=r_dram[0:1, R0 : R0 + RW].rearrange("a (p j) -> (a p) j", p=128),
                        in_=y,
                    )

            def emit_outproj(c):
                C0 = c * CHUNK
                ght = ghts.pop(c)
                rbc = osbp.tile([128, CHUNK], F32, tag="rbc", name="rbc", bufs=o["rbcb"])
                nc.sync.dma_start(
                    out=rbc, in_=r_dram[0:1, C0 : C0 + CHUNK].to_broadcast([128, CHUNK])
                )
                for t in range(2):
                    nc.vector.tensor_mul(ght[:, t, :], ght[:, t, :], rbc)
                for mt in range(HID // 128):
                    pj = psA.tile([128, CHUNK], F32, tag="pj", name="pjo")
                    for kt in range(2):
                        nc.tensor.matmul(
                            pj,
                            lhsT=wo_sb[:, kt, mt * 128 : (mt + 1) * 128],
                            rhs=ght[:, kt, :],
                            start=(kt == 0),
                            stop=(kt == 1),
                        )
                    osb = osbp.tile([128, CHUNK], BF16, tag="osb", name="osb")
                    if mt % 2 == 0:
                        nc.scalar.copy(osb, pj)
                    else:
                        nc.vector.tensor_copy(osb, pj)
                    nc.sync.dma_start(
                        out=outp_d[mt * 128 : (mt + 1) * 128, C0 : C0 + CHUNK],
                        in_=osb,
                    )

            LAG = o["lag"]
            wo_loaded = False
            done = 0  # next outproj chunk to emit
            hst_next = (hst0, hst1)
            for s in range(NCHUNK // 2):
                cA, cB = 2 * s, 2 * s + 1
                hsts = hst_next
                cg = emit_proj_sweep([cA, cB], hsts)
                if s + 1 < NCHUNK // 2:
                    hst_next = (load_hst(2 * s + 2), load_hst(2 * s + 3))
                if not wo_loaded:
                    nc.sync.dma_start(
                        out=wo_sb, in_=wo_d[:].rearrange("(kt p) m -> p kt m", p=128)
                    )
                    wo_loaded = True
                emit_attention(cA, *cg[0])
                emit_attention(cB, *cg[1])
                while done < min(cB + 1 - LAG, cB):
                    emit_outproj(done)
                    done += 1
            while done < NCHUNK:
                emit_outproj(done)
                done += 1

    nc.finalize()
    return nc


def _prep_inputs(hidden_states, kv_cache, W_qkv, W_gate, W_out, norm_weight):
    hsT = np.ascontiguousarray(hidden_states.T).astype(BF)
    in_maps = []
    arr = np.arange(BLOCK, dtype=np.float32) + 1.0  # 1..256
    nloc = np.arange(BLOCK, dtype=np.float32)
    for c in range(NCORES):
        heads = [4 * c + h for h in range(HL)]
        # fused weight: [Q(4x64), K(4x64), V(4x64), gate(256)] x HID
        rows = []
        for part in range(3):  # q, k, v
            for g in heads:
                base = g * 3 * D + part * D
                rows.append(W_qkv[base : base + D])
        rows.append(W_gate[c * 256 : (c + 1) * 256])
        w_comb = np.concatenate(rows, axis=0)  # [1024, HID]
        # mt-major SBUF image: [mt, p, kt*128+m] with element = W_combT[kt*128+p, mt*128+m]
        wcomb = np.ascontiguousarray(
            w_comb.T.reshape(HID // 128, 128, 8, 128).transpose(2, 1, 0, 3).reshape(8, 128, HID)
        ).astype(BF)

        # 0.5 factor: gate sigmoid computed on-device as tanh-based 1+tanh(x/2)
        w_out_c = (
            W_out[:, c * 256 : (c + 1) * 256]
            * norm_weight[c * 256 : (c + 1) * 256][None, :]
            * 0.5
        )
        wout = np.ascontiguousarray(w_out_c.T).astype(BF)  # [256, HID]

        s = SLOPE[heads]  # [4]
        qd = np.zeros((128, HL // 2, BLOCK), np.float32)
        kdb = np.zeros((128, HL // 2, BLOCK), np.float32)
        dd = np.zeros((128, 2 * HL, BLOCK), np.float32)
        bd = np.zeros((128, HL // 2), np.float32)
        kv0 = np.zeros((128, HL // 2, D), np.float32)
        for h in range(HL):
            sh = s[h]
            b = (h % 2) * 64
            p = h // 2
            qd[b : b + 64, p, :] = np.exp(-sh * arr)[None, :]
            kdb[b : b + 64, p, :] = np.exp(-sh * (BLOCK - nloc - 1))[None, :]
            bd[b : b + 64, p] = math.exp(-sh * BLOCK)
            kv0[b : b + 64, p, :] = kv_cache[heads[h]]
            for half in range(2):
                npos = half * 128 + nloc[:128]
                idx = arr[None, :] - 1 - npos[:, None]  # m - n
                dd[:, 2 * h + half, :] = np.where(idx >= 0, np.exp(-sh * idx), 0.0)
        in_maps.append(
            {
                "hsT": hsT,
                "wcomb": wcomb,
                "wout": wout,
                "dd": dd.astype(BF),
                "qd": qd.astype(BF),
                "kdb": kdb.astype(BF),
                "bd": bd,
                "kv0": kv0,
            }
        )
    return in_maps


def kernel(**inputs):
    global _NC_CACHE, LAST_EXEC_NS, LAST_RESULTS
    hidden_states = np.asarray(inputs["hidden_states"], dtype=np.float32)
    kv_cache = np.asarray(inputs["kv_cache"], dtype=np.float32)
    W_qkv = np.asarray(inputs["W_qkv"], dtype=np.float32)
    W_gate = np.asarray(inputs["W_gate"], dtype=np.float32)
    W_out = np.asarray(inputs["W_out"], dtype=np.float32)
    norm_weight = np.asarray(inputs["norm_weight"], dtype=np.float32)

    if _NC_CACHE is None:
        _NC_CACHE = _build_module()
    nc = _NC_CACHE

    in_maps = _prep_inputs(hidden_states, kv_cache, W_qkv, W_gate, W_out, norm_weight)
    res = run_bass_kernel_spmd(nc, in_maps, core_ids=list(range(NCORES)))
    LAST_EXEC_NS = res.exec_time_ns
    LAST_RESULTS = res
    acc = res.results[0]["outp"].astype(np.float64)
    for c in range(1, NCORES):
        acc += res.results[c]["outp"].astype(np.float64)
    return np.ascontiguousarray(acc.T).astype(np.float32)


# revision 13
# speedup vs baseline: 1.0663x; 1.0663x over previous
"""MiniMaxText01 linear attention layer on 8 Trainium2 NeuronCores.

Tensor-parallel over heads (4 heads per core). Per core:
  - fused QKV+gate projection in transposed layout (features on
    partitions, sequence on free dim), bf16 matmuls into 2-bank PSUM
    tiles (two m-tiles per tile) so the SiLU/Tanh drains run as one
    [128,1024] ACT op each; gate sigmoid computed as 0.5*(1+tanh(x/2))
    so every ACT function (Silu/Tanh/Square/Copy) shares one LUT set
  - lightning (chunked linear) attention with per-head decay, two heads
    packed per 128-partition group; per block the k'/v PE-transposes
    land in ONE [128,512] PSUM bank drained by a single DVE copy, and
    both head-pairs' attention outputs land in ONE [128,512] bank so
    the square/gate-mul drains are single 512-col ops
  - RMSNorm variance: ones-matmul partition reduction, per-chunk 2KB
    AllReduce across the 8 cores, DVE-only Newton rsqrt; out-projection
    for chunk c runs LAG chunks behind the attention pipeline so the
    collective latency is hidden
  - out-proj row-parallel: each core emits a full-width partial output
    (transposed, bf16); host sums the 8 partials and transposes back.

Everything is hardcoded for the fixed problem shapes below.
"""

import math
import warnings

warnings.filterwarnings("ignore")

import numpy as np
import ml_dtypes

import concourse.bacc as bacc
import concourse.mybir as mybir
import concourse.tile as tile
from concourse.bass_utils import run_bass_kernel_spmd
from concourse.masks import make_identity

F32 = mybir.dt.float32
I32 = mybir.dt.int32
BF16 = mybir.dt.bfloat16
BF = ml_dtypes.bfloat16
AF = mybir.ActivationFunctionType
ALU = mybir.AluOpType

N = 8192          # sequence length
HID = 2048        # hidden size
H = 32            # total heads
D = 64            # head dim
BLOCK = 256       # attention chunk size
NCORES = 8
HL = H // NCORES  # 4 local heads per core
CHUNK = 512       # seq columns processed per projection chunk
NCHUNK = N // CHUNK
BPC = CHUNK // BLOCK  # blocks per chunk
EPS = 1e-5
NUM_LAYERS, LAYER_IDX = 80, 0
MAGIC = 0x5F3759DF

LAST_EXEC_NS = None
LAST_RESULTS = None


def _build_slopes(n):
    def p2(m):
        start = 2 ** (-(2 ** (-(math.log2(m) - 3))))
        return [start * start**i for i in range(m)]

    if math.log2(n).is_integer():
        s = p2(n)
    else:
        cp = 2 ** math.floor(math.log2(n))
        s = p2(cp) + _build_slopes(2 * cp).tolist()[0::2][: n - cp]
    return np.array(s, dtype=np.float32)


SLOPE = _build_slopes(H) * (1.0 - LAYER_IDX / (NUM_LAYERS - 1) + 1e-5)  # [H]

_NC_CACHE = None


DEFAULT_OPTS = dict(
    psA=3, psB=2, psC=1, psV=2,
    hstb=4, combb=2, gateb=2, ghb=5, kvb=4, kptb=3, qpb=3, qkpb=3, hsqb=3, osbb=6,
    lag=2, cc_stride=1, rbcb=2, rcpb=2, warmup=48,
)


def _build_module(**opts):
    o = dict(DEFAULT_OPTS)
    o.update(opts)
    nc = bacc.Bacc("TRN2", target_bir_lowering=False, num_devices=NCORES)

    hsT_d = nc.dram_tensor("hsT", [HID, N], BF16, kind="ExternalInput")
    wc_d = nc.dram_tensor("wcomb", [8, 128, HID], BF16, kind="ExternalInput")
    wo_d = nc.dram_tensor("wout", [2 * 128, HID], BF16, kind="ExternalInput")
    dd_d = nc.dram_tensor("dd", [128, 2 * HL, BLOCK], BF16, kind="ExternalInput")
    qd_d = nc.dram_tensor("qd", [128, HL // 2, BLOCK], BF16, kind="ExternalInput")
    kdb_d = nc.dram_tensor("kdb", [128, HL // 2, BLOCK], BF16, kind="ExternalInput")
    bd_d = nc.dram_tensor("bd", [128, HL // 2], F32, kind="ExternalInput")
    kv0_d = nc.dram_tensor("kv0", [128, HL // 2, D], F32, kind="ExternalInput")
    outp_d = nc.dram_tensor("outp", [HID, N], BF16, kind="ExternalOutput")

    with tile.TileContext(nc) as tc:
        with (
            tc.tile_pool(name="singles", bufs=1) as sg,
            tc.tile_pool(name="hstp", bufs=o["hstb"]) as hstp,
            tc.tile_pool(name="combp", bufs=o["combb"]) as combp,
            tc.tile_pool(name="gatep", bufs=o["gateb"]) as gatep,
            tc.tile_pool(name="ghp", bufs=o["ghb"]) as ghp,
            tc.tile_pool(name="kvp", bufs=o["kvb"]) as kvp,
            tc.tile_pool(name="kptp", bufs=o["kptb"]) as kptp,
            tc.tile_pool(name="qpp", bufs=o["qpb"]) as qpp,
            tc.tile_pool(name="qkpp", bufs=o["qkpb"]) as qkpp,
            tc.tile_pool(name="hsqp", bufs=o["hsqb"]) as hsqp,
            tc.tile_pool(name="osbp", bufs=o["osbb"]) as osbp,
            tc.tile_pool(name="rcp", bufs=o["rcpb"]) as rcp,
            tc.tile_pool(name="psA", bufs=o["psA"], space="PSUM") as psA,
            tc.tile_pool(name="psB", bufs=o["psB"], space="PSUM") as psB,
            tc.tile_pool(name="psC", bufs=o["psC"], space="PSUM") as psC,
            tc.tile_pool(name="psV", bufs=o["psV"], space="PSUM") as psV,
            tc.tile_pool(name="dram", bufs=1, space="DRAM") as dram,
        ):
            # ---- resident tensors -------------------------------------
            wcm = []
            for mt in range(8):
                wct = sg.tile([128, HID // 128, 128], BF16, name=f"wcm{mt}")
                wcm.append(wct)
            wo_sb = sg.tile([128, 2, HID], BF16)
            dd_sb = sg.tile([128, 2 * HL, BLOCK], BF16)
            qd_sb = sg.tile([128, HL // 2, BLOCK], BF16)
            kdb_sb = sg.tile([128, HL // 2, BLOCK], BF16)
            bd_sb = sg.tile([128, HL // 2], F32)
            S32 = sg.tile([128, HL // 2, D], F32)
            Sbf = sg.tile([128, HL // 2, D], BF16)
            ones_sb = sg.tile([128, 1], BF16)
            nc.vector.memset(ones_sb, 1.0)
            ident = sg.tile([128, 128], BF16)
            make_identity(nc, ident)

            cc_in = dram.tile([1, N], F32)
            cc_out = dram.tile([1, N], F32)
            r_dram = dram.tile([1, N], F32)

            hsT_r = hsT_d[:].rearrange("(kt p) s -> p kt s", p=128)

            ghts = {}

            def load_hst(c):
                C0 = c * CHUNK
                hst_lo = hstp.tile([128, HID // 256, CHUNK], BF16, name="hst_lo", tag="hst_lo")
                hst_hi = hstp.tile([128, HID // 256, CHUNK], BF16, name="hst_hi", tag="hst_hi")
                nc.sync.dma_start(out=hst_lo, in_=hsT_r[:, 0 : HID // 256, C0 : C0 + CHUNK])
                nc.sync.dma_start(out=hst_hi, in_=hsT_r[:, HID // 256 :, C0 : C0 + CHUNK])
                return hst_lo, hst_hi

            # chunk 0+1 activations and the first weight m-tiles go out
            # first (critical path of the first projection group); the rest
            # spreads over the scalar/gpsimd DMA queues.
            hst0 = load_hst(0)
            nc.sync.dma_start(
                out=wcm[0], in_=wc_d[0].rearrange("p (kt m) -> p kt m", m=128)
            )
            hst1 = load_hst(1)
            for mt in range(1, 8):
                eng = nc.scalar if mt < 5 else nc.gpsimd
                eng.dma_start(
                    out=wcm[mt], in_=wc_d[mt].rearrange("p (kt m) -> p kt m", m=128)
                )
            nc.gpsimd.dma_start(out=dd_sb, in_=dd_d[:])
            nc.gpsimd.dma_start(out=qd_sb, in_=qd_d[:])
            nc.gpsimd.dma_start(out=kdb_sb, in_=kdb_d[:])
            nc.gpsimd.dma_start(out=bd_sb, in_=bd_d[:])
            nc.gpsimd.dma_start(out=S32, in_=kv0_d[:])
            nc.vector.tensor_copy(Sbf[:], S32[:])

            # PE warm-up during the initial DMA: junk transposes so the HAM
            # clock-gate reaches 8/8 before the first real matmul.
            warm = psB.tile([128, 512], BF16, tag="qk", name="warm")
            for i in range(o["warmup"]):
                nc.tensor.transpose(
                    warm[:, (i % 4) * 128 : (i % 4 + 1) * 128], in_=ident, identity=ident
                )

            def emit_proj(c, hst_pre=None):
                hst_lo, hst_hi = hst_pre if hst_pre is not None else load_hst(c)
                comb = combp.tile([128, 6, CHUNK], BF16, name="comb")
                gate = gatep.tile([128, 2, CHUNK], F32, name="gate")
                for mt in range(8):
                    pj = psA.tile([128, CHUNK], F32, tag="pj", name="pj")
                    for kt in range(HID // 128):
                        hsth = hst_lo if kt < HID // 256 else hst_hi
                        nc.tensor.matmul(
                            pj,
                            lhsT=wcm[mt][:, kt, :],
                            rhs=hsth[:, kt % (HID // 256), :],
                            start=(kt == 0),
                            stop=(kt == HID // 128 - 1),
                        )
                    if mt < 6:
                        nc.scalar.activation(comb[:, mt, :], pj, AF.Silu)
                    else:
                        # sigmoid(x) = 0.5*(1 + tanh(x/2)); the 0.5 is folded
                        # into W_out on the host, the +1 is applied below.
                        nc.scalar.activation(gate[:, mt - 6, :], pj, AF.Tanh, scale=0.5)
                nc.vector.tensor_scalar_add(gate[:], gate[:], 1.0)
                return comb, gate

            def emit_attention(c, comb, gate):
                C0 = c * CHUNK
                ght = ghp.tile([128, 2, CHUNK], BF16, name="ght", tag="ght")
                ghts[c] = ght

                ps_var = psV.tile([1, CHUNK], F32, tag="var", name="ps_var")

                for blk in range(BPC):
                    bc = blk * BLOCK
                    kvs = {}
                    # phase A: k-decay pre-scale + PE-transpose k', v into
                    # one [128,512] bf16 bank, single DVE drain
                    for p in range(2):
                        kpt = kptp.tile([128, BLOCK], BF16, tag="kpt", name="kpt")
                        nc.vector.tensor_mul(
                            kpt, comb[:, 2 + p, bc : bc + BLOCK], kdb_sb[:, p, :]
                        )
                        tp = psB.tile([128, 512], BF16, tag="qk", name="tp")
                        for half in range(2):
                            nc.tensor.transpose(
                                tp[:, half * 128 : (half + 1) * 128],
                                in_=kpt[:, half * 128 : (half + 1) * 128],
                                identity=ident,
                            )
                            nc.tensor.transpose(
                                tp[:, 256 + half * 128 : 256 + (half + 1) * 128],
                                in_=comb[:, 4 + p, bc + half * 128 : bc + (half + 1) * 128],
                                identity=ident,
                            )
                        kvt = kvp.tile([128, 4, 128], BF16, tag="kvt", name="kvt")
                        kvs[p] = kvt
                        nc.vector.tensor_copy(
                            kvt[:].rearrange("p a b -> p (a b)"), tp[:]
                        )
                    # phase B: decayed queries (both heads of a pair at once)
                    qps = {}
                    for p in range(2):
                        qp = qpp.tile([128, BLOCK], BF16, tag="qp", name="qp")
                        qps[p] = qp
                        nc.vector.tensor_mul(
                            qp, comb[:, p, bc : bc + BLOCK], qd_sb[:, p, :]
                        )
                    # phase C: scores (transposed) + decay mask
                    qkp = {}
                    for p in range(2):
                        for hi in range(2):
                            h = 2 * p + hi
                            b = hi * 64
                            qkph = qkpp.tile([128, 2, BLOCK], BF16, tag="qkp", name="qkph")
                            qkp[h] = qkph
                            qk_ps = psB.tile([128, 512], F32, tag="qk", name="qk_ps")
                            for half in range(2):
                                nc.tensor.matmul(
                                    qk_ps[:, half * 256 : (half + 1) * 256],
                                    lhsT=comb[b : b + 64, 2 + p, bc + half * 128 : bc + (half + 1) * 128],
                                    rhs=comb[b : b + 64, p, bc : bc + BLOCK],
                                    start=True,
                                    stop=True,
                                    tile_position=(b, 0),
                                )
                            nc.vector.tensor_mul(
                                qkph[:].rearrange("p a b -> p (a b)"),
                                qk_ps,
                                dd_sb[:, 2 * h : 2 * h + 2, :].rearrange("p a b -> p (a b)"),
                            )
                    # phase D: attention output (transposed): inter + intra,
                    # both head-pairs into one [128,512] bank
                    po = psC.tile([128, 2, BLOCK], F32, tag="po", name="po")
                    for p in range(2):
                        for hi in range(2):
                            b = hi * 64
                            nc.tensor.matmul(
                                po[b : b + 64, p, :],
                                lhsT=Sbf[b : b + 64, p, :],
                                rhs=qps[p][b : b + 64, :],
                                start=True,
                                stop=False,
                                tile_position=(b, b),
                            )
                        for hi in range(2):
                            b = hi * 64
                            for half in range(2):
                                nc.tensor.matmul(
                                    po[b : b + 64, p, :],
                                    lhsT=kvs[p][:, 2 + half, b : b + 64],
                                    rhs=qkp[2 * p + hi][:, half, :],
                                    start=False,
                                    stop=(half == 1),
                                    tile_position=(0, b),
                                )
                    # phase E: variance partials + gated hidden (single ops)
                    hsq = hsqp.tile([128, 2, BLOCK], BF16, tag="hsq", name="hsq")
                    nc.scalar.square(hsq[:].rearrange("p a b -> p (a b)"),
                                     po[:].rearrange("p a b -> p (a b)"))
                    for p in range(2):
                        nc.tensor.matmul(
                            ps_var[0:1, bc : bc + BLOCK],
                            lhsT=ones_sb,
                            rhs=hsq[:, p, :],
                            start=(p == 0),
                            stop=(p == 1),
                        )
                    nc.vector.tensor_mul(
                        ght[:, :, bc : bc + BLOCK],
                        po[:],
                        gate[:, :, bc : bc + BLOCK],
                    )
                    # phase F: state update S = bd*S + k'^T v
                    for p in range(2):
                        psS = psB.tile([128, D], F32, tag="qk", name="psS")
                        for hi in range(2):
                            b = hi * 64
                            for half in range(2):
                                nc.tensor.matmul(
                                    psS[b : b + 64, :],
                                    lhsT=kvs[p][:, half, b : b + 64],
                                    rhs=kvs[p][:, 2 + half, b : b + 64],
                                    start=(half == 0),
                                    stop=(half == 1),
                                    tile_position=(0, b),
                                )
                        nc.scalar.mul(S32[:, p, :], S32[:, p, :], bd_sb[:, p : p + 1])
                        nc.vector.tensor_add(S32[:, p, :], S32[:, p, :], psS)
                    nc.vector.tensor_copy(Sbf[:], S32[:])

                # variance all-reduce + Newton rsqrt (DVE only)
                ssqc = hsqp.tile([1, CHUNK], F32, tag="ssqc", name="ssqc")
                nc.scalar.copy(ssqc, ps_var)
                nc.scalar.dma_start(out=cc_in[0:1, C0 : C0 + CHUNK], in_=ssqc)
                stride = o["cc_stride"]
                if (c + 1) % stride == 0:
                    R0 = (c + 1 - stride) * CHUNK
                    RW = stride * CHUNK
                    nc.gpsimd.collective_compute(
                        "AllReduce",
                        mybir.AluOpType.add,
                        replica_groups=[list(range(NCORES))],
                        ins=[cc_in[0:1, R0 : R0 + RW].opt()],
                        outs=[cc_out[0:1, R0 : R0 + RW].opt()],
                    )
                    z = rcp.tile([128, RW // 128], F32, tag="z", name="z")
                    nc.gpsimd.dma_start(
                        out=z,
                        in_=cc_out[0:1, R0 : R0 + RW].rearrange("a (p j) -> (a p) j", p=128),
                    )
                    nc.vector.tensor_scalar(
                        out=z, in0=z, scalar1=1.0 / HID, scalar2=EPS, op0=ALU.mult, op1=ALU.add
                    )
                    yi = rcp.tile([128, RW // 128], I32, tag="yi", name="yi")
                    nc.vector.tensor_scalar(
                        out=yi, in0=z.bitcast(I32), scalar1=1, scalar2=None,
                        op0=ALU.logical_shift_right,
                    )
                    nc.vector.tensor_scalar(
                        out=yi, in0=yi, scalar1=-1, scalar2=MAGIC, op0=ALU.mult, op1=ALU.add
                    )
                    y = yi.bitcast(F32)
                    t = rcp.tile([128, RW // 128], F32, tag="t", name="t")
                    for _ in range(2):
                        nc.vector.tensor_mul(t, y, y)
                        nc.vector.tensor_mul(t, t, z)
                        nc.vector.tensor_scalar(
                            out=t, in0=t, scalar1=-0.5, scalar2=1.5, op0=ALU.mult, op1=ALU.add
                        )
                        nc.vector.tensor_mul(y, y, t)
                    nc.gpsimd.dma_start(
                        out=r_dram[0:1, R0 : R0 + RW].rearrange("a (p j) -> (a p) j", p=128),
                        in_=y,
                    )

            def emit_outproj(c):
                C0 = c * CHUNK
                ght = ghts.pop(c)
                rbc = osbp.tile([128, CHUNK], F32, tag="rbc", name="rbc", bufs=o["rbcb"])
                nc.sync.dma_start(
                    out=rbc, in_=r_dram[0:1, C0 : C0 + CHUNK].to_broadcast([128, CHUNK])
                )
                for t in range(2):
                    nc.vector.tensor_mul(ght[:, t, :], ght[:, t, :], rbc)
                for mt in range(HID // 128):
                    pj = psA.tile([128, CHUNK], F32, tag="pj", name="pjo")
                    for kt in range(2):
                        nc.tensor.matmul(
                            pj,
                            lhsT=wo_sb[:, kt, mt * 128 : (mt + 1) * 128],
                            rhs=ght[:, kt, :],
                            start=(kt == 0),
                            stop=(kt == 1),
                        )
                    osb = osbp.tile([128, CHUNK], BF16, tag="osb", name="osb")
                    if mt % 2 == 0:
                        nc.scalar.copy(osb, pj)
                    else:
                        nc.vector.tensor_copy(osb, pj)
                    nc.sync.dma_start(
                        out=outp_d[mt * 128 : (mt + 1) * 128, C0 : C0 + CHUNK],
                        in_=osb,
                    )

            LAG = o["lag"]
            wo_loaded = False
            done = 0  # next outproj chunk to emit
            hst_next = {0: hst0, 1: hst1}
            for c in range(NCHUNK):
                cg = emit_proj(c, hst_pre=hst_next.pop(c, None))
                if c + 2 < NCHUNK and (c + 2) not in hst_next:
                    hst_next[c + 2] = load_hst(c + 2)
                if not wo_loaded:
                    nc.sync.dma_start(
                        out=wo_sb, in_=wo_d[:].rearrange("(kt p) m -> p kt m", p=128)
                    )
                    wo_loaded = True
                emit_attention(c, *cg)
                while done < min(c + 1 - LAG, c):
                    emit_outproj(done)
                    done += 1
            while done < NCHUNK:
                emit_outproj(done)
                done += 1

    nc.finalize()
    return nc


def _prep_inputs(hidden_states, kv_cache, W_qkv, W_gate, W_out, norm_weight):
    hsT = np.ascontiguousarray(hidden_states.T).astype(BF)
    in_maps = []
    arr = np.arange(BLOCK, dtype=np.float32) + 1.0  # 1..256
    nloc = np.arange(BLOCK, dtype=np.float32)
    for c in range(NCORES):
        heads = [4 * c + h for h in range(HL)]
        # fused weight: [Q(4x64), K(4x64), V(4x64), gate(256)] x HID
        rows = []
        for part in range(3):  # q, k, v
            for g in heads:
                base = g * 3 * D + part * D
                rows.append(W_qkv[base : base + D])
        rows.append(W_gate[c * 256 : (c + 1) * 256])
        w_comb = np.concatenate(rows, axis=0)  # [1024, HID]
        # mt-major SBUF image: [mt, p, kt*128+m] with element = W_combT[kt*128+p, mt*128+m]
        wcomb = np.ascontiguousarray(
            w_comb.T.reshape(HID // 128, 128, 8, 128).transpose(2, 1, 0, 3).reshape(8, 128, HID)
        ).astype(BF)

        # 0.5 factor: gate sigmoid computed on-device as tanh-based 1+tanh(x/2)
        w_out_c = (
            W_out[:, c * 256 : (c + 1) * 256]
            * norm_weight[c * 256 : (c + 1) * 256][None, :]
            * 0.5
        )
        wout = np.ascontiguousarray(w_out_c.T).astype(BF)  # [256, HID]

        s = SLOPE[heads]  # [4]
        qd = np.zeros((128, HL // 2, BLOCK), np.float32)
        kdb = np.zeros((128, HL // 2, BLOCK), np.float32)
        dd = np.zeros((128, 2 * HL, BLOCK), np.float32)
        bd = np.zeros((128, HL // 2), np.float32)
        kv0 = np.zeros((128, HL // 2, D), np.float32)
        for h in range(HL):
            sh = s[h]
            b = (h % 2) * 64
            p = h // 2
            qd[b : b + 64, p, :] = np.exp(-sh * arr)[None, :]
            kdb[b : b + 64, p, :] = np.exp(-sh * (BLOCK - nloc - 1))[None, :]
            bd[b : b + 64, p] = math.exp(-sh * BLOCK)
            kv0[b : b + 64, p, :] = kv_cache[heads[h]]
            for half in range(2):
                npos = half * 128 + nloc[:128]
                idx = arr[None, :] - 1 - npos[:, None]  # m - n
                dd[:, 2 * h + half, :] = np.where(idx >= 0, np.exp(-sh * idx), 0.0)
        in_maps.append(
            {
                "hsT": hsT,
                "wcomb": wcomb,
                "wout": wout,
                "dd": dd.astype(BF),
                "qd": qd.astype(BF),
                "kdb": kdb.astype(BF),
                "bd": bd,
                "kv0": kv0,
            }
        )
    return in_maps


def kernel(**inputs):
    global _NC_CACHE, LAST_EXEC_NS, LAST_RESULTS
    hidden_states = np.asarray(inputs["hidden_states"], dtype=np.float32)
    kv_cache = np.asarray(inputs["kv_cache"], dtype=np.float32)
    W_qkv = np.asarray(inputs["W_qkv"], dtype=np.float32)
    W_gate = np.asarray(inputs["W_gate"], dtype=np.float32)
    W_out = np.asarray(inputs["W_out"], dtype=np.float32)
    norm_weight = np.asarray(inputs["norm_weight"], dtype=np.float32)

    if _NC_CACHE is None:
        _NC_CACHE = _build_module()
    nc = _NC_CACHE

    in_maps = _prep_inputs(hidden_states, kv_cache, W_qkv, W_gate, W_out, norm_weight)
    res = run_bass_kernel_spmd(nc, in_maps, core_ids=list(range(NCORES)))
    LAST_EXEC_NS = res.exec_time_ns
    LAST_RESULTS = res
    acc = res.results[0]["outp"].astype(np.float64)
    for c in range(1, NCORES):
        acc += res.results[c]["outp"].astype(np.float64)
    return np.ascontiguousarray(acc.T).astype(np.float32)
